# revision 1
# baseline (speedup 1.0000x reference)
"""Mixtral attention layer (B=1, S=2048, H=4096, NH=32, NKV=8, HD=128) on 8
Trainium2 NeuronCores, tensor-parallel over heads.

Sharding: core c owns 4 query heads + 1 KV head (column-shard of wq/wk/wv,
row-shard of wo).  Each core computes a full [S, H] partial of the o_proj
output; the host sums the 8 partials and adds the residual (the gather of a
row-parallel matmul).

Per-core pipeline (projection/attention matmuls in float32r = fp22-truncated
fp32, full PE rate at N>=256):
  Pass A (norm stats): x^T streamed as bf16; ACT squares it, a ones-vector
    matmul reduces sum(x^2) over H (partition reduction on PE) into PSUM;
    r = 1/sqrt(mean+eps) is partition-broadcast (GPSIMD) and folded into
    full-width RoPE cos/sin tables.
  Pass B (projections): x^T re-streamed in fp32r; 6 accumulating matmuls
    per H-chunk produce q^T (4 heads) / k^T / v^T in 6 PSUM banks; the PSUM
    evacuation applies norm + RoPE in 4 tensor ops per tile (DVE for q,
    GPSIMD for k/v).  norm_w is folded into the weights on the host.
  Attention: per head-pair sweep (both heads share this core's single KV
    head - GQA), causal flash-style: scores^T = k^T.T @ q^T chunkwise, exp
    on ACT (PSUM->SBUF), causal mask via GPSIMD affine_select on diagonal
    blocks, unnormalized AV + ones-matmul row-sum Z accumulate in PSUM; 1/Z
    applied at AV evacuation into SBUF-resident attn^T.
  o_proj: attn^T @ wo accumulated over the 4 heads, DMA'd out per tile.

q^T is spilled to internal DRAM between phases (SBUF pressure); attn^T
reuses the wk/wv SBUF slots after the projections retire.
"""

import math

import numpy as np

import concourse.bass as bass
import concourse.tile as tile
from concourse import bacc, mybir
from concourse.masks import make_identity

F32 = mybir.dt.float32
F32R = mybir.dt.float32r
BF16 = mybir.dt.bfloat16

# Full problem dims
B, S, H, NH, NKV, HD = 1, 2048, 4096, 32, 8, 128
EPS = 1e-5
N_CORES = 8
QH = NH // N_CORES          # query heads per core = 4
DQ = QH * HD                # q columns per core = 512
DKV = (NKV // N_CORES) * HD  # kv columns per core = 128


def build_bass(s=S, h=H, qh=QH, stop_after=None, diag=None):
    """Build the single-core Bass module (same NEFF on all 8 cores)."""
    ST = 512 if s >= 512 else s       # s-tile width (proj + attention i-tiles)
    NST = s // ST                     # number of s-tiles
    HC = h // 128                     # H contraction chunks
    NJ = s // 128                     # j chunks (keys)
    NSC = s // 128                    # s chunks for o_proj
    NHT = h // 512 if h >= 512 else 1  # h tiles for o_proj output
    HT = min(512, h)
    dq = qh * HD
    scale = 1.0 / math.sqrt(HD)

    nc = bacc.Bacc(None, target_bir_lowering=False)

    xT = nc.dram_tensor("xT", [h, s], F32R, kind="ExternalInput")
    xTb = nc.dram_tensor("xTb", [h, s], BF16, kind="ExternalInput")
    wq = nc.dram_tensor("wq", [h, dq], F32R, kind="ExternalInput")
    wk = nc.dram_tensor("wk", [h, DKV], F32R, kind="ExternalInput")
    wv = nc.dram_tensor("wv", [h, DKV], F32R, kind="ExternalInput")
    wo = nc.dram_tensor("wo", [dq, h], F32R, kind="ExternalInput")
    cosT = nc.dram_tensor("cosT", [HD, s], F32, kind="ExternalInput")
    sinTs = nc.dram_tensor("sinTs", [HD, s], F32, kind="ExternalInput")
    out = nc.dram_tensor("out", [s, h], F32, kind="ExternalOutput")

    xT_t = xT.rearrange("(ho hi) s -> hi ho s", hi=128)
    xTb_t = xTb.rearrange("(ho hi) s -> hi ho s", hi=128)
    wq_t = wq.rearrange("(ho hi) d -> hi ho d", hi=128)
    wk_t = wk.rearrange("(ho hi) d -> hi ho d", hi=128)
    wv_t = wv.rearrange("(ho hi) d -> hi ho d", hi=128)
    wo_t = wo.rearrange("(do di) h -> di do h", di=128)

    with tile.TileContext(nc) as tc:
        with (
            tc.tile_pool(name="persist", bufs=1) as persist,
            tc.tile_pool(name="xin", bufs=4) as xin,
            tc.tile_pool(name="xbin", bufs=3) as xbin,
            tc.tile_pool(name="x2b", bufs=3) as x2b,
            tc.tile_pool(name="rope", bufs=3) as ropep,
            tc.tile_pool(name="statp", bufs=4) as statp,
            tc.tile_pool(name="tabp", bufs=2) as tabp,
            tc.tile_pool(name="bcastp", bufs=3) as bcastp,
            tc.tile_pool(name="probs", bufs=6) as probs,
            tc.tile_pool(name="outp", bufs=3) as outp,
            tc.tile_pool(name="qin", bufs=3) as qin,
            tc.tile_pool(name="dramp", bufs=1, space="DRAM") as dramp,
            tc.tile_pool(name="acc_ps", bufs=8, space="PSUM") as acc_ps,
        ):
            # ---- persistent SBUF tensors ----
            # Slot reuse chains (same tag, sequential lifetimes):
            #   wq (8MB) -> wo (8MB)         tag "bigw"
            #   wk (2MB) -> attnT heads 0-1  tag "wk"
            #   wv (2MB) -> attnT heads 2-3  tag "wv"
            #   cos (1MB) -> v natural (1MB) tag "cosvnat"
            wq_sb = persist.tile([128, HC, dq], F32R, tag="bigw")
            wk_sb = persist.tile([128, HC, DKV], F32R, tag="wk")
            wv_sb = persist.tile([128, HC, DKV], F32R, tag="wv")
            cos_sb = persist.tile([128, s], F32, tag="cosvnat")
            sin_sb = persist.tile([128, s], F32, tag="sin")
            ones_f = persist.tile([128, 1], F32, tag="ones_f")
            ones_sb = persist.tile([128, 1], F32R, tag="ones")
            ones_bf = persist.tile([128, 1], BF16, tag="ones_bf")
            eps_sb = persist.tile([1, 1], F32, tag="eps")
            ident_sb = persist.tile([128, 128], F32, tag="ident")
            kT_sb = persist.tile([128, s], F32R, tag="kT")
            vT_sb = persist.tile([128, s], F32, tag="vT")
            # q^T spilled to DRAM, re-streamed by attention
            qT_dr = dramp.tile([128, qh, s], F32R, tag="qT_dr")

            nc.sync.dma_start(out=wq_sb, in_=wq_t)
            nc.sync.dma_start(out=wk_sb, in_=wk_t)
            nc.sync.dma_start(out=wv_sb, in_=wv_t)
            nc.sync.dma_start(out=cos_sb, in_=cosT[:, :])
            nc.sync.dma_start(out=sin_sb, in_=sinTs[:, :])
            nc.vector.memset(ones_f, 1.0)
            nc.scalar.copy(ones_sb, ones_f)
            nc.scalar.copy(ones_bf, ones_f)
            nc.vector.memset(eps_sb, EPS)
            make_identity(nc, ident_sb)

            # ---- phase 1: interleaved pass A (norm stats, bf16) and
            # pass B (q/k/v projections, fp32r), pass A one s-tile ahead ----
            def pass_a(st):
                ss = bass.ts(st, ST)
                sq_ps = acc_ps.tile([1, ST], F32, tag="acc", name="sq_ps")
                for hc in range(HC):
                    xb_sb = xbin.tile([128, ST], BF16)
                    nc.sync.dma_start(out=xb_sb, in_=xTb_t[:, hc, ss])
                    x2_sb = x2b.tile([128, ST], BF16)
                    nc.scalar.square(x2_sb, xb_sb)
                    nc.tensor.matmul(sq_ps, ones_bf, x2_sb,
                                     start=(hc == 0), stop=(hc == HC - 1))
                # r = 1/sqrt(mean + eps); fold into cos/sin tables
                sd_sb = statp.tile([1, ST], F32, tag="stat", name="sd_sb")
                nc.scalar.activation(
                    sd_sb, sq_ps, mybir.ActivationFunctionType.Sqrt,
                    bias=eps_sb, scale=1.0 / h,
                )
                rr_sb = statp.tile([1, ST], F32, tag="stat", name="rr_sb")
                nc.vector.reciprocal(rr_sb, sd_sb)
                R_t = tabp.tile([128, ST], F32, tag="R", name="R_t")
                nc.gpsimd.partition_broadcast(R_t, rr_sb)
                cp_t = tabp.tile([128, ST], F32, tag="cp", name="cp_t")
                nc.vector.tensor_mul(cp_t, cos_sb[:, ss], R_t)
                sp_t = tabp.tile([128, ST], F32, tag="sp", name="sp_t")
                nc.vector.tensor_mul(sp_t, sin_sb[:, ss], R_t)
                return R_t, cp_t, sp_t

            def pass_b(st, tabs):
                R_t, cp_t, sp_t = tabs
                ss = bass.ts(st, ST)
                q_ps = [acc_ps.tile([128, ST], F32, tag="acc", name=f"q_ps{m}")
                        for m in range(qh)]
                k_ps = acc_ps.tile([128, ST], F32, tag="acc", name="k_ps")
                v_ps = acc_ps.tile([128, ST], F32, tag="acc", name="v_ps")
                for hc in range(HC):
                    x_sb = xin.tile([128, ST], F32R)
                    nc.sync.dma_start(out=x_sb, in_=xT_t[:, hc, ss])
                    st_, sp_ = (hc == 0), (hc == HC - 1)
                    for m in range(qh):
                        nc.tensor.matmul(
                            q_ps[m], wq_sb[:, hc, bass.ts(m, 128)], x_sb,
                            start=st_, stop=sp_,
                        )
                    nc.tensor.matmul(k_ps, wk_sb[:, hc, :], x_sb,
                                     start=st_, stop=sp_)
                    nc.tensor.matmul(v_ps, wv_sb[:, hc, :], x_sb,
                                     start=st_, stop=sp_)
                # evacuation: fast ACT copy frees the PSUM bank, then
                # norm+RoPE happens SBUF-side on DVE (in place; the u-halves
                # read the raw values before the cos-multiply overwrites)
                def rope_evac(src_ps, dst):
                    u_sb = ropep.tile([128, ST], F32, tag="u", name="u_sb",
                                      bufs=2)
                    nc.scalar.copy(dst, src_ps)
                    nc.vector.tensor_mul(
                        u_sb[0:64, :], dst[64:128, :], sp_t[64:128, :])
                    nc.vector.tensor_mul(
                        u_sb[64:128, :], dst[0:64, :], sp_t[0:64, :])
                    nc.vector.tensor_mul(dst, dst, cp_t)
                    nc.vector.tensor_add(dst, dst, u_sb)

                for m in range(qh if diag != "no_evac" else 0):
                    dst = ropep.tile([128, ST], F32R, tag="t", name="t_sb",
                                     bufs=4)
                    rope_evac(q_ps[m], dst)
                    nc.sync.dma_start(out=qT_dr[:, m, ss], in_=dst)
                if diag == "no_evac":
                    return
                rope_evac(k_ps, kT_sb[:, ss])
                nc.scalar.copy(vT_sb[:, ss], v_ps)
                nc.vector.tensor_mul(vT_sb[:, ss], vT_sb[:, ss], R_t)

            if diag == "no_pa":
                R_t = tabp.tile([128, ST], F32, tag="R", name="R_t")
                cp_t = tabp.tile([128, ST], F32, tag="cp", name="cp_t")
                sp_t = tabp.tile([128, ST], F32, tag="sp", name="sp_t")
                nc.vector.memset(R_t, 1.0)
                nc.vector.memset(cp_t, 1.0)
                nc.vector.memset(sp_t, 1.0)
                for st in range(NST):
                    pass_b(st, (R_t, cp_t, sp_t))
            else:
                tabs = pass_a(0)
                for st in range(NST):
                    pass_b(st, tabs)
                    if st + 1 < NST:
                        tabs = pass_a(st + 1)

            # ---- phase 2: transpose v to natural [j, d] layout ----
            vnat_sb = persist.tile([128, NJ, 128], F32R, tag="cosvnat")
            wo_sb = persist.tile([128, qh, h], F32R, tag="bigw")
            if stop_after != "p1":
                nc.sync.dma_start(out=wo_sb, in_=wo_t)
            for jc in range(NJ if stop_after != "p1" else 0):
                vt_ps = acc_ps.tile([128, 128], F32, tag="acc")
                nc.tensor.transpose(vt_ps, vT_sb[:, bass.ts(jc, 128)], ident_sb)
                nc.scalar.copy(vnat_sb[:, jc, :], vt_ps)

            # attn^T reuses the wk/wv slots (heads 0-1 / 2-3)
            attnT_h = [
                persist.tile([128, 2, s], F32R, tag="wk", name="attnT01"),
                persist.tile([128, 2, s], F32R, tag="wv", name="attnT23"),
            ]

            def attn_slice(m, sl):
                return attnT_h[m // 2][:, m % 2, sl]

            # ---- phase 3 + 4 interleaved: attention per i-tile (both
            # head pairs), then immediately the o_proj matmuls for that
            # i-range so they fill PE stalls in the next i-tile's attention
            def attn_tile(hp, ti):
                heads = (2 * hp, 2 * hp + 1)
                if True:
                    iss = bass.ts(ti, ST)
                    q_sbs = []
                    for hh in heads:
                        q_sb = qin.tile([128, ST], F32R, tag="q",
                                        name=f"q_sb{hh}")
                        nc.sync.dma_start(out=q_sb, in_=qT_dr[:, hh, iss])
                        q_sbs.append(q_sb)
                    av_ps = [acc_ps.tile([128, ST], F32, tag="acc",
                                         name=f"av_ps{i}") for i in range(2)]
                    z_ps = [acc_ps.tile([1, ST], F32, tag="acc",
                                        name=f"z_ps{i}") for i in range(2)]
                    njc = (ti + 1) * (ST // 128)
                    for jc in range(njc):
                        st_, sp_ = (jc == 0), (jc == njc - 1)
                        diag = (jc + 1) * 128 > ti * ST
                        for i in range(2):
                            s_ps = acc_ps.tile([128, ST], F32, tag="acc",
                                               name=f"s_ps{i}")
                            nc.tensor.matmul(
                                s_ps, kT_sb[:, bass.ts(jc, 128)], q_sbs[i],
                                start=True, stop=True,
                            )
                            p_sb = probs.tile([128, ST], F32R, tag="p",
                                              name=f"p_sb{i}", bufs=6)
                            nc.scalar.activation(
                                p_sb, s_ps, mybir.ActivationFunctionType.Exp,
                                scale=scale,
                            )
                            if diag:
                                nc.gpsimd.affine_select(
                                    out=p_sb, in_=p_sb,
                                    pattern=[[1, ST]],
                                    compare_op=mybir.AluOpType.is_ge,
                                    fill=0.0,
                                    base=ti * ST - jc * 128,
                                    channel_multiplier=-1,
                                )
                            nc.tensor.matmul(av_ps[i], vnat_sb[:, jc, :], p_sb,
                                             start=st_, stop=sp_)
                            nc.tensor.matmul(z_ps[i], ones_sb, p_sb,
                                             start=st_, stop=sp_)
                    for i, hh in enumerate(heads):
                        zr_sb = statp.tile([1, ST], F32, tag="stat",
                                           name="zr_sb")
                        nc.vector.reciprocal(zr_sb, z_ps[i])
                        ZR_sb = bcastp.tile([128, ST], F32, tag="bcast",
                                            name="ZR_sb")
                        nc.gpsimd.partition_broadcast(ZR_sb, zr_sb)
                        nc.vector.tensor_mul(attn_slice(hh, iss), av_ps[i],
                                             ZR_sb)

            def o_proj_chunk(sc):
                scs = bass.ts(sc, 128)
                for ht in range(NHT):
                    o_ps = acc_ps.tile([128, HT], F32, tag="acc")
                    for m in range(qh):
                        nc.tensor.matmul(
                            o_ps, attn_slice(m, scs),
                            wo_sb[:, m, bass.ts(ht, HT)],
                            start=(m == 0), stop=(m == qh - 1),
                        )
                    o_sb = outp.tile([128, HT], F32)
                    if (sc + ht) % 2 == 0:
                        nc.scalar.copy(o_sb, o_ps)
                    else:
                        nc.vector.tensor_copy(o_sb, o_ps)
                    dma_eng = nc.sync
                    dma_eng.dma_start(
                        out=out[scs, bass.ts(ht, HT)], in_=o_sb
                    )

            if stop_after not in ("p1", "p2"):
                for ti in range(NST):
                    for hp in range(qh // 2):
                        attn_tile(hp, ti)
                    if stop_after is None:
                        for sc in range(ti * (ST // 128), (ti + 1) * (ST // 128)):
                            o_proj_chunk(sc)

    nc.compile()
    return nc


def make_core_inputs(hidden_states, cos, sin, norm_w, wq, wk, wv, wo,
                     s=S, h=H, qh=QH, n_cores=N_CORES):
    """Host-side sharding + layout preparation. Returns list of in_maps."""
    import ml_dtypes

    dq = qh * HD
    dkv = DKV
    x = np.asarray(hidden_states, dtype=np.float32).reshape(s, h)
    nw = np.asarray(norm_w, dtype=np.float32)
    xT = np.ascontiguousarray(x.T)                      # [h, s]
    xTb = np.ascontiguousarray(xT.astype(ml_dtypes.bfloat16))
    cosT = np.ascontiguousarray(np.asarray(cos, np.float32).reshape(s, HD).T)
    sinT = np.ascontiguousarray(np.asarray(sin, np.float32).reshape(s, HD).T)
    # swapped/sign-flipped sin table: rows 0:64 = +sin_half, 64:128 = -sin_half
    sin_half = sinT[0:64]
    sinTs = np.ascontiguousarray(np.concatenate([sinT[64:128], -sin_half], axis=0))
    # fold norm_w into the projection weights
    wq_f = np.asarray(wq, np.float32) * nw[:, None]
    wk_f = np.asarray(wk, np.float32) * nw[:, None]
    wv_f = np.asarray(wv, np.float32) * nw[:, None]
    wo_f = np.asarray(wo, np.float32)

    in_maps = []
    for c in range(n_cores):
        in_maps.append({
            "xT": xT,
            "xTb": xTb,
            "wq": np.ascontiguousarray(wq_f[:, c * dq:(c + 1) * dq]),
            "wk": np.ascontiguousarray(wk_f[:, c * dkv:(c + 1) * dkv]),
            "wv": np.ascontiguousarray(wv_f[:, c * dkv:(c + 1) * dkv]),
            "wo": np.ascontiguousarray(wo_f[c * dq:(c + 1) * dq, :]),
            "cosT": cosT,
            "sinTs": sinTs,
        })
    return in_maps


_NC_CACHE = {}


def kernel(hidden_states, cos, sin, norm_w, wq, wk, wv, wo):
    from concourse.bass_utils import run_bass_kernel_spmd

    if "nc" not in _NC_CACHE:
        _NC_CACHE["nc"] = build_bass()
    nc = _NC_CACHE["nc"]
    in_maps = make_core_inputs(hidden_states, cos, sin, norm_w, wq, wk, wv, wo)
    res = run_bass_kernel_spmd(nc, in_maps, core_ids=list(range(N_CORES)))
    partials = [m["out"] for m in res.results]
    out = np.asarray(hidden_states, np.float32).reshape(S, H).copy()
    for p in partials:
        out += p
    return out.reshape(B, S, H)



# revision 43
# speedup vs baseline: 1.3380x; 1.3380x over previous
"""Mixtral attention layer (B=1, S=2048, H=4096, NH=32, NKV=8, HD=128) on 8
Trainium2 NeuronCores, tensor-parallel over heads.

Sharding: core c owns 4 query heads + 1 KV head (column-shard of wq/wk/wv,
row-shard of wo).  Each core computes a full [S, H] partial of the o_proj
output (bf16); the host sums the 8 partials and adds the residual.

Per-core pipeline (projection/attention matmuls in float32r = fp22-truncated
fp32, full PE rate at N>=256):
  Phase 1 (per 512-wide s-tile): x^T streamed fp32r once; per H-chunk, 6
    accumulating matmuls produce q^T (4 heads) / k^T / v^T in PSUM, and the
    norm stats ride the same stream (ACT squares into fp8e4, DoubleRow
    ones-matmul reduces sum(x^2) at 2x PE rate).  PSUM evacuation is split:
    plain ACT copies first (frees the banks for the next tile), then
    r = 1/sqrt(mean+eps) is folded into the RoPE tables and applied SBUF-side
    on DVE while the next tile's matmuls run.  v is transposed to natural
    [j, d] layout per tile (PE transpose via bf16 identity).  Weight DMAs are
    split fine (wk/wv halves, wq eighth-then-quarters) and priority-ordered
    so PE starts ~5us in; wo per-head loads chain into the wq slots.
  Attention: per (head-pair, i-tile), causal flash-style with a software-
    pipelined j-loop: scores for chunk j+1 issue before AV/Z of chunk j
    (PSUM tags: scores x3, AV x2, Z-pair x1, o_proj x2 = 8 banks).  exp on
    ACT, causal mask via GPSIMD affine_select on diagonal chunks only, with
    free-dim subranges (>=256 wide) on diagonal chunks to skip dead work.
    Row-sums Z via ones-matmul accumulate alongside AV (rows at partitions
    0/32 of one bank); 1/Z applied at AV evacuation into SBUF attn^T.
  o_proj: attn^T @ wo accumulated over the 4 heads, staged to bf16 rows and
    DMA'd out per 128-row chunk; emitted after each i-tile so the matmuls
    fill PE stalls in the next i-tile's attention.

q^T is spilled to internal DRAM between phases (SBUF pressure) and
prefetched one head-pair ahead during attention.  Slot reuse chains:
wq quarter -> wo per-head (2MB x4), wk -> attnT heads 0-1, wv -> attnT 2-3.
"""

import math

import numpy as np

import concourse.bass as bass
import concourse.tile as tile
from concourse import bacc, mybir
from concourse.masks import make_identity

F32 = mybir.dt.float32
F32R = mybir.dt.float32r
BF16 = mybir.dt.bfloat16
FP8 = mybir.dt.float8e4

# Full problem dims
B, S, H, NH, NKV, HD = 1, 2048, 4096, 32, 8, 128
EPS = 1e-5
N_CORES = 8
QH = NH // N_CORES          # query heads per core = 4
DQ = QH * HD                # q columns per core = 512
DKV = (NKV // N_CORES) * HD  # kv columns per core = 128

ST = 512                    # s-tile width
NST = S // ST               # 4
HC = H // 128               # 32 H-contraction chunks
G = 2                       # x chunks per DMA / squares pair


def build_bass(s=S, h=H, qh=QH):
    dq = qh * HD
    nst = s // ST
    scale = 1.0 / math.sqrt(HD)
    Exp = mybir.ActivationFunctionType.Exp

    nc = bacc.Bacc(None, target_bir_lowering=False)

    xT = nc.dram_tensor("xT", [h, s], F32R, kind="ExternalInput")
    wq = nc.dram_tensor("wq", [h, dq], F32R, kind="ExternalInput")
    wk = nc.dram_tensor("wk", [h, DKV], F32R, kind="ExternalInput")
    wv = nc.dram_tensor("wv", [h, DKV], F32R, kind="ExternalInput")
    wo = nc.dram_tensor("wo", [dq, h], F32R, kind="ExternalInput")
    cosT = nc.dram_tensor("cosT", [HD, s], F32, kind="ExternalInput")
    sinTs = nc.dram_tensor("sinTs", [HD, s], F32, kind="ExternalInput")
    out = nc.dram_tensor("out", [s, h], BF16, kind="ExternalOutput")

    xT_t = xT.rearrange("(ho hi) s -> hi ho s", hi=128)
    wq_t = wq.rearrange("(ho hi) d -> hi ho d", hi=128)
    wk_t = wk.rearrange("(ho hi) d -> hi ho d", hi=128)
    wv_t = wv.rearrange("(ho hi) d -> hi ho d", hi=128)
    wo_t = wo.rearrange("(do di) h -> di do h", di=128)

    with tile.TileContext(nc) as tc:
        with (
            tc.tile_pool(name="persist", bufs=1) as persist,
            tc.tile_pool(name="xin", bufs=4) as xin,
            tc.tile_pool(name="x2p", bufs=2) as x2p,
            tc.tile_pool(name="ropep", bufs=2) as ropep,
            tc.tile_pool(name="tabp", bufs=2) as tabp,
            tc.tile_pool(name="statp", bufs=2) as statp,
            tc.tile_pool(name="zrp", bufs=2) as zrp,
            tc.tile_pool(name="probs", bufs=4) as probs,
            tc.tile_pool(name="dramp", bufs=1, space="DRAM") as dramp,
            tc.tile_pool(name="acc_ps", bufs=8, space="PSUM") as ps,
        ):
            # ---- persistent SBUF ----
            # PSUM tags (8 banks): pA(3) q0-2 | scores; pB(2) q3,k | av;
            # pC(1) sum-sq | z-pair; pD(2) v,v-transpose | o_proj.
            wqq = [
                persist.tile([128, 8, dq], F32R, tag=f"bigw{i}", name=f"wqq{i}")
                for i in range(4)
            ]
            wk_sb = persist.tile([128, HC, DKV], F32R, tag="wk")
            wv_sb = persist.tile([128, HC, DKV], F32R, tag="wv")
            cos_sb = persist.tile([128, s], F32, tag="cos")
            sin_sb = persist.tile([128, s], F32, tag="sin")
            kT_sb = persist.tile([128, s], F32R, tag="kT")
            vT_sb = persist.tile([128, s], F32, tag="vT")
            vnat_sb = persist.tile([128, s // 128, 128], F32R, tag="vnat")
            qin_sb = persist.tile([128, 2, 2, ST], F32R, tag="qin")
            ones_f = persist.tile([128, 1], F32, tag="ones_f")
            ones_r = persist.tile([128, 1], F32R, tag="ones_r")
            ones_b = persist.tile([128, 1], BF16, tag="ones_b")
            ones8 = persist.tile([128, 2, 128], FP8, tag="ones8")
            ones128_f = persist.tile([128, 128], F32, tag="ones128")
            eps_sb = persist.tile([1, 1], F32, tag="eps")
            ident_sb = persist.tile([128, 128], F32, tag="ident")
            qT_dr = dramp.tile([128, qh, s], F32R, tag="qT_dr")

            nc.vector.memset(ones_f, 1.0)

            nc.scalar.copy(ones_b, ones_f)
            nc.vector.memset(ones128_f, 1.0)
            nc.scalar.copy(ones_r, ones128_f[:, 0:1])
            nc.scalar.copy(ones8[:, 0, :], ones128_f)
            nc.scalar.copy(ones8[:, 1, :], ones128_f)
            nc.vector.memset(eps_sb, EPS)
            make_identity(nc, ident_sb)

            # first-needed weight pieces lead the DMA queue; the x stream and
            # the weight tails interleave behind them (priority = emission)
            nc.sync.dma_start(out=wk_sb[:, 0:4, :], in_=wk_t[:, 0:4, :])
            nc.sync.dma_start(out=wv_sb[:, 0:4, :], in_=wv_t[:, 0:4, :])
            nc.sync.dma_start(out=wqq[0][:, 0:2, :], in_=wq_t[:, 0:2, :])

            # ---- phase 1 ----
            def pass_b(st, interleave=None):
                ss = bass.ts(st, ST)
                q_ps = [
                    ps.tile([128, ST], F32, tag=("pA" if m < 3 else "pB"),
                            bufs=(3 if m < 3 else 2), name=f"q_ps{m}")
                    for m in range(qh)
                ]
                k_ps = ps.tile([128, ST], F32, tag="pB", bufs=2, name="k_ps")
                v_ps = ps.tile([128, ST], F32, tag="pD", bufs=2, name="v_ps")
                sq = ps.tile([128, ST], F32, tag="pC", bufs=1, name="sq")
                for g in range(HC // G):
                    xg = xin.tile([128, G, ST], F32R, name="xg")
                    nc.sync.dma_start(out=xg, in_=xT_t[:, g * G:(g + 1) * G, ss])
                    x2 = x2p.tile([128, 2, ST], FP8, name="x2")
                    for cc in range(G):
                        hc = g * G + cc
                        x_sb = xg[:, cc, :]
                        st_, sp_ = (hc == 0), (hc == HC - 1)
                        qt, lo = hc // 8, hc % 8
                        nc.tensor.matmul(k_ps, wk_sb[:, hc, :], x_sb,
                                         start=st_, stop=sp_)
                        nc.tensor.matmul(v_ps, wv_sb[:, hc, :], x_sb,
                                         start=st_, stop=sp_)
                        for m in range(qh):
                            nc.tensor.matmul(
                                q_ps[m], wqq[qt][:, lo, bass.ts(m, 128)], x_sb,
                                start=st_, stop=sp_,
                            )
                        nc.scalar.square(x2[:, cc, :], x_sb)
                    # all-ones weight is invariant under the SwInterleave
                    # layout, so the 2x-rate dual-fp8 mode needs no repacking
                    nc.tensor.matmul(
                        sq, ones8, x2,
                        start=(g == 0), stop=(g == HC // G - 1),
                        perf_mode=mybir.MatmulPerfMode.DoubleRow,
                    )
                    if interleave is not None:
                        interleave(g)
                return q_ps, k_ps, v_ps, sq

            def evac(st, q_ps, k_ps, v_ps, sq, defer=False):
                ss = bass.ts(st, ST)
                # plain copies first, split ACT/DVE in the next tile's
                # chunk-0 matmul order (k,v,q0..q3): frees the PSUM banks
                # with minimal serial latency
                nc.vector.tensor_copy(kT_sb[:, ss], k_ps)
                nc.vector.tensor_copy(vT_sb[:, ss], v_ps)
                qds = []
                for m in range(qh):
                    qd = ropep.tile([128, ST], F32R, tag="qd", name="qd",
                                    bufs=4)
                    nc.scalar.copy(qd, q_ps[m])
                    qds.append(qd)
                # stats scalars immediately (frees the pC bank + stat slots)
                sd = statp.tile([1, ST], F32, tag="stat", name="sd")
                nc.scalar.activation(
                    sd, sq[0:1, :], mybir.ActivationFunctionType.Sqrt,
                    bias=eps_sb, scale=1.0 / h,
                )
                rr = statp.tile([1, ST], F32, tag="stat", name="rr")
                nc.vector.reciprocal(rr, sd)

                def rest():
                    # rope tables, then SBUF-side rotation on DVE
                    R = tabp.tile([128, ST], F32, tag="R", name="R")
                    nc.gpsimd.partition_broadcast(R, rr)
                    cp = tabp.tile([128, ST], F32, tag="cp", name="cp")
                    nc.vector.tensor_mul(cp, cos_sb[:, ss], R)
                    sp = tabp.tile([128, ST], F32, tag="sp", name="sp")
                    nc.vector.tensor_mul(sp, sin_sb[:, ss], R)
                    # v path first: the transposes recycle the pC bank for
                    # the next tile's stats accumulator (and the z-pair)
                    nc.vector.tensor_mul(vT_sb[:, ss], vT_sb[:, ss], R)
                    for j4 in range(ST // 128):
                        jc = st * (ST // 128) + j4
                        vt = ps.tile([128, 128], F32, tag="pC", bufs=1,
                                     name="vt")
                        nc.tensor.transpose(vt, vT_sb[:, bass.ts(jc, 128)],
                                            ident_sb)
                        nc.scalar.copy(vnat_sb[:, jc, :], vt)

                    def rope(dst):
                        u = ropep.tile([128, ST], F32, tag="u", name="u",
                                       bufs=1)
                        nc.vector.tensor_mul(u[0:64, :], dst[64:128, :],
                                             sp[64:128, :])
                        nc.vector.tensor_mul(u[64:128, :], dst[0:64, :],
                                             sp[0:64, :])
                        nc.vector.tensor_mul(dst, dst, cp)
                        nc.vector.tensor_add(dst, dst, u)

                    for m in range(qh):
                        rope(qds[m])
                        nc.sync.dma_start(out=qT_dr[:, m, ss], in_=qds[m])
                    rope(kT_sb[:, ss])

                if defer:
                    return rest
                rest()
                return None

            def _wq_piece(qt, a, b):
                nc.sync.dma_start(out=wqq[qt][:, a:b, :],
                                  in_=wq_t[:, 8 * qt + a:8 * qt + b, :])

            def _kv_piece(a, b):
                nc.sync.dma_start(out=wk_sb[:, a:b, :], in_=wk_t[:, a:b, :])
                nc.sync.dma_start(out=wv_sb[:, a:b, :], in_=wv_t[:, a:b, :])

            # need-ordered weight stream: one piece after each x group of
            # tile 0 (tile 0 is DMA-paced; later tiles have DMA slack)
            _pieces = [
                lambda: (_kv_piece(4, 8), _wq_piece(0, 2, 4)),
                lambda: _wq_piece(0, 4, 8),
                lambda: _kv_piece(8, 16),
                lambda: _wq_piece(1, 0, 3),
                lambda: _wq_piece(1, 3, 6),
                lambda: _wq_piece(1, 6, 8),
                lambda: _kv_piece(16, 24),
                lambda: _wq_piece(2, 0, 3),
                lambda: _wq_piece(2, 3, 6),
                lambda: _wq_piece(2, 6, 8),
                lambda: _kv_piece(24, 32),
                lambda: _wq_piece(3, 0, 3),
                lambda: _wq_piece(3, 3, 6),
                lambda: _wq_piece(3, 6, 8),
                lambda: nc.sync.dma_start(out=cos_sb, in_=cosT[:, :]),
                lambda: nc.sync.dma_start(out=sin_sb, in_=sinTs[:, :]),
            ]

            def interleave0(g):
                if g < len(_pieces):
                    _pieces[g]()

            def prefetch_q(pi):
                if pi >= 2 * nst:
                    return
                ti_, hp_ = pi // 2, pi % 2
                nc.sync.dma_start(
                    out=qin_sb[:, pi % 2],
                    in_=qT_dr[:, 2 * hp_:2 * hp_ + 2, bass.ts(ti_, ST)],
                )

            def _first_prefetches():
                # one pair ahead only: two-ahead would overwrite the slot the
                # current pair is still reading (same ring parity)
                prefetch_q(0)

            evac3_rest = None
            for st in range(nst):
                q_ps, k_ps, v_ps, sq = pass_b(
                    st, interleave0 if st == 0 else None)
                evac3_rest = evac(st, q_ps, k_ps, v_ps, sq,
                                  defer=(st == nst - 1))

            _first_prefetches()

            # ---- attention + o_proj ----

            # wo per-head loads, slot-chained behind the wq quarters; split
            # in halves so they don't starve the x stream in tile 3
            wom = []
            for m in range(qh):
                t = persist.tile([128, h], F32R, tag=f"bigw{m}", name=f"wom{m}")
                nc.sync.dma_start(out=t[:, 0:h // 2], in_=wo_t[:, m, 0:h // 2])
                nc.sync.dma_start(out=t[:, h // 2:h], in_=wo_t[:, m, h // 2:h])
                wom.append(t)

            attnT_h = [
                persist.tile([128, 2, s], F32R, tag="wk", name="attnT01"),
                persist.tile([128, 2, s], F32R, tag="wv", name="attnT23"),
            ]
            # o_proj staging ring reuses the sin-table slot (sin's last read
            # is at the end of phase 1)
            out_sb = persist.tile([128, 2, h // 2], BF16, tag="sin",
                                  name="out_sb")

            def attn_slice(m, sl):
                return attnT_h[m // 2][:, m % 2, sl]

            def attn_tile(hp, ti):
                pi = ti * 2 + hp
                slot = pi % 2
                iss = bass.ts(ti, ST)
                prefetch_q(pi + 1)
                njc = (ti + 1) * (ST // 128)
                # heads run sequentially: each gets the full z bank at
                # partition 0 (matmul dst must start at partition 0) and a
                # 2-deep scores pipeline in the three pA slots
                for i in range(2):
                    hh = 2 * hp + i
                    av = ps.tile([128, ST], F32, tag="pB", bufs=2,
                                 name=f"av{i}")
                    zz = ps.tile([1, ST], F32, tag="pC", bufs=1, name="zz")
                    sdict = {}

                    def emit_scores(jc):
                        i0 = min(max(jc * 128 - ti * ST, 0), ST - 256)
                        sp_ = ps.tile([128, ST], F32, tag="pA", bufs=3,
                                      name="s_ps")
                        nc.tensor.matmul(
                            sp_[:, i0:], kT_sb[:, bass.ts(jc, 128)],
                            qin_sb[:, slot, i, i0:], start=True, stop=True,
                        )
                        sdict[jc] = (sp_, i0)

                    emit_scores(0)
                    if njc > 1:
                        emit_scores(1)
                    for jc in range(njc):
                        if jc + 2 < njc:
                            emit_scores(jc + 2)
                        diag = (jc + 1) * 128 > ti * ST
                        sp_, i0 = sdict.pop(jc)
                        p = probs.tile([128, ST], F32R, tag="p", name="p",
                                       bufs=4)
                        nc.scalar.activation(p[:, i0:], sp_[:, i0:], Exp,
                                             scale=scale)
                        if diag:
                            nc.gpsimd.affine_select(
                                out=p[:, i0:], in_=p[:, i0:],
                                pattern=[[1, ST - i0]],
                                compare_op=mybir.AluOpType.is_ge,
                                fill=0.0,
                                base=ti * ST + i0 - jc * 128,
                                channel_multiplier=-1,
                            )
                        st_, sp_f = (jc == 0), (jc == njc - 1)
                        nc.tensor.matmul(av[:, i0:], vnat_sb[:, jc, :],
                                         p[:, i0:], start=st_, stop=sp_f)
                        nc.tensor.matmul(zz[0:1, i0:], ones_r,
                                         p[:, i0:], start=st_, stop=sp_f)
                    zr = statp.tile([1, ST], F32, tag="zrs", name="zr")
                    nc.vector.reciprocal(zr, zz[0:1, :])
                    ZR = zrp.tile([128, ST], F32, tag="zr", name="ZR")
                    nc.gpsimd.partition_broadcast(ZR, zr)
                    nc.vector.tensor_mul(attn_slice(hh, iss), av, ZR)

            def o_proj_chunk(sc):
                scs = bass.ts(sc, 128)
                for half in range(2):
                    ot = out_sb[:, (sc * 2 + half) % 2]
                    for hq in range(4):
                        ht = half * 4 + hq
                        o_ps = ps.tile([128, 512], F32, tag="pD", bufs=2,
                                       name="o_ps")
                        for m in range(qh):
                            nc.tensor.matmul(
                                o_ps, attn_slice(m, scs),
                                wom[m][:, bass.ts(ht, 512)],
                                start=(m == 0), stop=(m == qh - 1),
                            )
                        dst = ot[:, bass.ts(hq, 512)]
                        if (sc + ht) % 2 == 0:
                            nc.scalar.copy(dst, o_ps)
                        else:
                            nc.vector.tensor_copy(dst, o_ps)
                    nc.sync.dma_start(
                        out=out[scs, bass.ts(half, h // 2)], in_=ot
                    )

            BISECT = False
            if BISECT:
                for ti in range(nst):
                    attn_tile(0, ti)
                    attn_tile(1, ti)
                    for sc in range(ti * (ST // 128), (ti + 1) * (ST // 128)):
                        o_proj_chunk(sc)
            else:
                # o_proj chunks are deferred one i-tile and dribbled out at
                # the attention seams, where the ZR-evac chains would
                # otherwise leave PE idle
                pending = []
                for ti in range(nst):
                    for hp in range(2):
                        attn_tile(hp, ti)
                        if evac3_rest is not None:
                            # the last tile's rope/v work only feeds the
                            # ti=3 pairs; emit it behind the first pair
                            evac3_rest()
                            evac3_rest = None
                        for _ in range(2):
                            if pending:
                                o_proj_chunk(pending.pop(0))
                    pending.extend(
                        range(ti * (ST // 128), (ti + 1) * (ST // 128)))
                for sc in pending:
                    o_proj_chunk(sc)

    nc.compile()
    return nc


def make_core_inputs(hidden_states, cos, sin, norm_w, wq, wk, wv, wo,
                     s=S, h=H, qh=QH, n_cores=N_CORES):
    """Host-side sharding + layout preparation. Returns list of in_maps."""
    dq = qh * HD
    dkv = DKV
    x = np.asarray(hidden_states, dtype=np.float32).reshape(s, h)
    nw = np.asarray(norm_w, dtype=np.float32)
    xT = np.ascontiguousarray(x.T)                      # [h, s]
    cosT = np.ascontiguousarray(np.asarray(cos, np.float32).reshape(s, HD).T)
    sinT = np.ascontiguousarray(np.asarray(sin, np.float32).reshape(s, HD).T)
    # swapped/sign-flipped sin table: rows 0:64 = +sin_half, 64:128 = -sin_half
    sin_half = sinT[0:64]
    sinTs = np.ascontiguousarray(np.concatenate([sinT[64:128], -sin_half], axis=0))
    # fold norm_w into the projection weights
    wq_f = np.asarray(wq, np.float32) * nw[:, None]
    wk_f = np.asarray(wk, np.float32) * nw[:, None]
    wv_f = np.asarray(wv, np.float32) * nw[:, None]
    wo_f = np.asarray(wo, np.float32)

    in_maps = []
    for c in range(n_cores):
        in_maps.append({
            "xT": xT,
            "wq": np.ascontiguousarray(wq_f[:, c * dq:(c + 1) * dq]),
            "wk": np.ascontiguousarray(wk_f[:, c * dkv:(c + 1) * dkv]),
            "wv": np.ascontiguousarray(wv_f[:, c * dkv:(c + 1) * dkv]),
            "wo": np.ascontiguousarray(wo_f[c * dq:(c + 1) * dq, :]),
            "cosT": cosT,
            "sinTs": sinTs,
        })
    return in_maps


_NC_CACHE = {}


def kernel(hidden_states, cos, sin, norm_w, wq, wk, wv, wo):
    from concourse.bass_utils import run_bass_kernel_spmd

    if "nc" not in _NC_CACHE:
        _NC_CACHE["nc"] = build_bass()
    nc = _NC_CACHE["nc"]
    in_maps = make_core_inputs(hidden_states, cos, sin, norm_w, wq, wk, wv, wo)
    res = run_bass_kernel_spmd(nc, in_maps, core_ids=list(range(N_CORES)))
    out = np.asarray(hidden_states, np.float32).reshape(S, H).copy()
    for m in res.results:
        out += np.asarray(m["out"], dtype=np.float32)
    return out.reshape(B, S, H)


# revision 51
# speedup vs baseline: 1.3389x; 1.0007x over previous
"""Mixtral attention layer (B=1, S=2048, H=4096, NH=32, NKV=8, HD=128) on 8
Trainium2 NeuronCores, tensor-parallel over heads.

Sharding: core c owns 4 query heads + 1 KV head (column-shard of wq/wk/wv,
row-shard of wo).  Each core computes a full [S, H] partial of the o_proj
output (bf16); the host sums the 8 partials and adds the residual.

Per-core pipeline (projection/attention matmuls in float32r = fp22-truncated
fp32, full PE rate at N>=256):
  Phase 1 (per 512-wide s-tile): x^T streamed fp32r once; per H-chunk, 6
    accumulating matmuls produce q^T (4 heads) / k^T / v^T in PSUM, and the
    norm stats ride the same stream (ACT squares into fp8e4; a dual-fp8
    DoubleRow ones-matmul reduces sum(x^2) at 2x PE rate).  PSUM evacuation
    is split: plain copies first on DVE (k,v) + ACT (q0-3) free the banks
    for the next tile with minimal latency; then r = 1/sqrt(mean+eps) is
    folded into the RoPE tables and applied SBUF-side on DVE while the next
    tile's matmuls run; v is transposed to natural [j, d] layout (PE
    transpose).  The last tile's table/rope/v work is deferred behind the
    first attention pair (only the ti=3 pairs consume it).  Weight DMAs are
    split fine (wk/wv quarters, wq 2-3 chunk pieces) and priority-ordered in
    need order so PE starts ~5us in; wo per-head loads chain into the wq
    slots and split in halves to avoid starving the tile-3 x stream.
  Attention: per (head-pair, i-tile), heads sequential, causal flash-style
    with a software-pipelined j-loop: scores run 2-3 chunks ahead of AV/Z
    (PSUM tags: scores x3, AV x2, Z x1, o_proj x2 = 8 banks; matmul dst
    must start at partition 0, hence one z bank per head in turn).  exp on
    ACT, causal mask via GPSIMD affine_select on diagonal chunks only, with
    free-dim subranges (>=256 wide, keeping fp32r full rate) on diagonal
    chunks to skip dead work.  Row-sum Z via ones-matmul accumulates
    alongside AV; 1/Z applied at AV evacuation into SBUF attn^T.
  o_proj: attn^T @ wo accumulated over the 4 heads, staged to bf16 rows and
    DMA'd out per 128-row half-chunk; chunks are deferred one i-tile and
    dribbled out at the attention seams so the matmuls fill the ZR-chain
    and head-switch stalls.

q^T is spilled to internal DRAM between phases (SBUF pressure) and
prefetched one head-pair ahead during attention (the 2-slot ring parity
forbids deeper prefetch).  Slot reuse chains: wq quarter -> wo per-head
(2MB x4), wk -> attnT heads 0-1, wv -> attnT 2-3, sin -> o_proj staging.
"""

import math

import numpy as np

import concourse.bass as bass
import concourse.tile as tile
from concourse import bacc, mybir
from concourse.masks import make_identity

F32 = mybir.dt.float32
F32R = mybir.dt.float32r
BF16 = mybir.dt.bfloat16
FP8 = mybir.dt.float8e4

# Full problem dims
B, S, H, NH, NKV, HD = 1, 2048, 4096, 32, 8, 128
EPS = 1e-5
N_CORES = 8
QH = NH // N_CORES          # query heads per core = 4
DQ = QH * HD                # q columns per core = 512
DKV = (NKV // N_CORES) * HD  # kv columns per core = 128

ST = 512                    # s-tile width
NST = S // ST               # 4
HC = H // 128               # 32 H-contraction chunks
G = 2                       # x chunks per DMA / squares pair


def build_bass(s=S, h=H, qh=QH):
    dq = qh * HD
    nst = s // ST
    scale = 1.0 / math.sqrt(HD)
    Exp = mybir.ActivationFunctionType.Exp

    nc = bacc.Bacc(None, target_bir_lowering=False)

    xT = nc.dram_tensor("xT", [h, s], F32R, kind="ExternalInput")
    wq = nc.dram_tensor("wq", [h, dq], F32R, kind="ExternalInput")
    wk = nc.dram_tensor("wk", [h, DKV], F32R, kind="ExternalInput")
    wv = nc.dram_tensor("wv", [h, DKV], F32R, kind="ExternalInput")
    wo = nc.dram_tensor("wo", [dq, h], F32R, kind="ExternalInput")
    cosT = nc.dram_tensor("cosT", [HD, s], F32, kind="ExternalInput")
    sinTs = nc.dram_tensor("sinTs", [HD, s], F32, kind="ExternalInput")
    out = nc.dram_tensor("out", [s, h], BF16, kind="ExternalOutput")

    xT_t = xT.rearrange("(ho hi) s -> hi ho s", hi=128)
    wq_t = wq.rearrange("(ho hi) d -> hi ho d", hi=128)
    wk_t = wk.rearrange("(ho hi) d -> hi ho d", hi=128)
    wv_t = wv.rearrange("(ho hi) d -> hi ho d", hi=128)
    wo_t = wo.rearrange("(do di) h -> di do h", di=128)

    with tile.TileContext(nc) as tc:
        with (
            tc.tile_pool(name="persist", bufs=1) as persist,
            tc.tile_pool(name="xin", bufs=4) as xin,
            tc.tile_pool(name="x2p", bufs=2) as x2p,
            tc.tile_pool(name="ropep", bufs=2) as ropep,
            tc.tile_pool(name="tabp", bufs=2) as tabp,
            tc.tile_pool(name="statp", bufs=2) as statp,
            tc.tile_pool(name="zrp", bufs=2) as zrp,
            tc.tile_pool(name="probs", bufs=4) as probs,
            tc.tile_pool(name="dramp", bufs=1, space="DRAM") as dramp,
            tc.tile_pool(name="acc_ps", bufs=8, space="PSUM") as ps,
        ):
            # ---- persistent SBUF ----
            # PSUM tags (8 banks): pA(3) q0-2 | scores; pB(2) q3,k | av;
            # pC(1) sum-sq | z-pair; pD(2) v,v-transpose | o_proj.
            wqq = [
                persist.tile([128, 8, dq], F32R, tag=f"bigw{i}", name=f"wqq{i}")
                for i in range(4)
            ]
            wk_sb = persist.tile([128, HC, DKV], F32R, tag="wk")
            wv_sb = persist.tile([128, HC, DKV], F32R, tag="wv")
            cos_sb = persist.tile([128, s], F32, tag="cos")
            sin_sb = persist.tile([128, s], F32, tag="sin")
            kT_sb = persist.tile([128, s], F32R, tag="kT")
            vT_sb = persist.tile([128, s], F32, tag="vT")
            vnat_sb = persist.tile([128, s // 128, 128], F32R, tag="vnat")
            qin_sb = persist.tile([128, 2, 2, ST], F32R, tag="qin")
            ones_f = persist.tile([128, 1], F32, tag="ones_f")
            ones_r = persist.tile([128, 1], F32R, tag="ones_r")
            ones_b = persist.tile([128, 1], BF16, tag="ones_b")
            ones8 = persist.tile([128, 2, 128], FP8, tag="ones8")
            ones128_f = persist.tile([128, 128], F32, tag="ones128")
            eps_sb = persist.tile([1, 1], F32, tag="eps")
            ident_sb = persist.tile([128, 128], F32, tag="ident")
            qT_dr = dramp.tile([128, qh, s], F32R, tag="qT_dr")

            nc.vector.memset(ones_f, 1.0)

            nc.scalar.copy(ones_b, ones_f)
            nc.vector.memset(ones128_f, 1.0)
            nc.scalar.copy(ones_r, ones128_f[:, 0:1])
            nc.scalar.copy(ones8[:, 0, :], ones128_f)
            nc.scalar.copy(ones8[:, 1, :], ones128_f)
            nc.vector.memset(eps_sb, EPS)
            make_identity(nc, ident_sb)

            # first-needed weight pieces lead the DMA queue; the x stream and
            # the weight tails interleave behind them (priority = emission)
            nc.sync.dma_start(out=wk_sb[:, 0:4, :], in_=wk_t[:, 0:4, :])
            nc.sync.dma_start(out=wv_sb[:, 0:4, :], in_=wv_t[:, 0:4, :])
            nc.sync.dma_start(out=wqq[0][:, 0:2, :], in_=wq_t[:, 0:2, :])

            # ---- phase 1 ----
            def pass_b(st, interleave=None):
                ss = bass.ts(st, ST)
                q_ps = [
                    ps.tile([128, ST], F32, tag=("pA" if m < 3 else "pB"),
                            bufs=(3 if m < 3 else 2), name=f"q_ps{m}")
                    for m in range(qh)
                ]
                k_ps = ps.tile([128, ST], F32, tag="pB", bufs=2, name="k_ps")
                v_ps = ps.tile([128, ST], F32, tag="pD", bufs=2, name="v_ps")
                sq = ps.tile([128, ST], F32, tag="pC", bufs=1, name="sq")
                for g in range(HC // G):
                    xg = xin.tile([128, G, ST], F32R, name="xg")
                    nc.sync.dma_start(out=xg, in_=xT_t[:, g * G:(g + 1) * G, ss])
                    x2 = x2p.tile([128, 2, ST], FP8, name="x2")
                    for cc in range(G):
                        hc = g * G + cc
                        x_sb = xg[:, cc, :]
                        st_, sp_ = (hc == 0), (hc == HC - 1)
                        qt, lo = hc // 8, hc % 8
                        nc.tensor.matmul(k_ps, wk_sb[:, hc, :], x_sb,
                                         start=st_, stop=sp_)
                        nc.tensor.matmul(v_ps, wv_sb[:, hc, :], x_sb,
                                         start=st_, stop=sp_)
                        for m in range(qh):
                            nc.tensor.matmul(
                                q_ps[m], wqq[qt][:, lo, bass.ts(m, 128)], x_sb,
                                start=st_, stop=sp_,
                            )
                        nc.scalar.square(x2[:, cc, :], x_sb)
                    # all-ones weight is invariant under the SwInterleave
                    # layout, so the 2x-rate dual-fp8 mode needs no repacking
                    nc.tensor.matmul(
                        sq, ones8, x2,
                        start=(g == 0), stop=(g == HC // G - 1),
                        perf_mode=mybir.MatmulPerfMode.DoubleRow,
                    )
                    if interleave is not None:
                        interleave(g)
                return q_ps, k_ps, v_ps, sq

            def evac(st, q_ps, k_ps, v_ps, sq, defer=False):
                ss = bass.ts(st, ST)
                # plain copies first, split ACT/DVE in the next tile's
                # chunk-0 matmul order (k,v,q0..q3): frees the PSUM banks
                # with minimal serial latency
                nc.vector.tensor_copy(kT_sb[:, ss], k_ps)
                nc.vector.tensor_copy(vT_sb[:, ss], v_ps)
                qds = []
                for m in range(qh):
                    qd = ropep.tile([128, ST], F32R, tag="qd", name="qd",
                                    bufs=4)
                    nc.scalar.copy(qd, q_ps[m])
                    qds.append(qd)
                # stats scalars immediately (frees the pC bank + stat slots)
                sd = statp.tile([1, ST], F32, tag="stat", name="sd")
                nc.scalar.activation(
                    sd, sq[0:1, :], mybir.ActivationFunctionType.Sqrt,
                    bias=eps_sb, scale=1.0 / h,
                )
                rr = statp.tile([1, ST], F32, tag="stat", name="rr")
                nc.vector.reciprocal(rr, sd)

                def rest():
                    # rope tables, then SBUF-side rotation on DVE
                    R = tabp.tile([128, ST], F32, tag="R", name="R")
                    nc.gpsimd.partition_broadcast(R, rr)
                    cp = tabp.tile([128, ST], F32, tag="cp", name="cp")
                    nc.vector.tensor_mul(cp, cos_sb[:, ss], R)
                    sp = tabp.tile([128, ST], F32, tag="sp", name="sp")
                    nc.vector.tensor_mul(sp, sin_sb[:, ss], R)
                    # v path first: the transposes recycle the pC bank for
                    # the next tile's stats accumulator (and the z-pair)
                    nc.vector.tensor_mul(vT_sb[:, ss], vT_sb[:, ss], R)
                    for j4 in range(ST // 128):
                        jc = st * (ST // 128) + j4
                        vt = ps.tile([128, 128], F32, tag="pC", bufs=1,
                                     name="vt")
                        nc.tensor.transpose(vt, vT_sb[:, bass.ts(jc, 128)],
                                            ident_sb)
                        nc.scalar.copy(vnat_sb[:, jc, :], vt)

                    def rope(dst):
                        u = ropep.tile([128, ST], F32, tag="u", name="u",
                                       bufs=1)
                        nc.vector.tensor_mul(u[0:64, :], dst[64:128, :],
                                             sp[64:128, :])
                        nc.vector.tensor_mul(u[64:128, :], dst[0:64, :],
                                             sp[0:64, :])
                        nc.vector.tensor_mul(dst, dst, cp)
                        nc.vector.tensor_add(dst, dst, u)

                    for m in range(qh):
                        rope(qds[m])
                        nc.sync.dma_start(out=qT_dr[:, m, ss], in_=qds[m])
                    rope(kT_sb[:, ss])

                if defer:
                    return rest
                rest()
                return None

            def _wq_piece(qt, a, b):
                nc.sync.dma_start(out=wqq[qt][:, a:b, :],
                                  in_=wq_t[:, 8 * qt + a:8 * qt + b, :])

            def _kv_piece(a, b):
                nc.sync.dma_start(out=wk_sb[:, a:b, :], in_=wk_t[:, a:b, :])
                nc.sync.dma_start(out=wv_sb[:, a:b, :], in_=wv_t[:, a:b, :])

            # need-ordered weight stream: one piece after each x group of
            # tile 0 (tile 0 is DMA-paced; later tiles have DMA slack)
            _pieces = [
                lambda: (_kv_piece(4, 8), _wq_piece(0, 2, 4)),
                lambda: _wq_piece(0, 4, 8),
                lambda: _kv_piece(8, 16),
                lambda: _wq_piece(1, 0, 3),
                lambda: _wq_piece(1, 3, 6),
                lambda: _wq_piece(1, 6, 8),
                lambda: _kv_piece(16, 24),
                lambda: _wq_piece(2, 0, 3),
                lambda: _wq_piece(2, 3, 6),
                lambda: _wq_piece(2, 6, 8),
                lambda: _kv_piece(24, 32),
                lambda: _wq_piece(3, 0, 3),
                lambda: _wq_piece(3, 3, 6),
                lambda: _wq_piece(3, 6, 8),
                lambda: nc.sync.dma_start(out=cos_sb, in_=cosT[:, :]),
                lambda: nc.sync.dma_start(out=sin_sb, in_=sinTs[:, :]),
            ]

            def interleave0(g):
                if g < len(_pieces):
                    _pieces[g]()

            def prefetch_q(pi):
                if pi >= 2 * nst:
                    return
                ti_, hp_ = pi // 2, pi % 2
                nc.sync.dma_start(
                    out=qin_sb[:, pi % 2],
                    in_=qT_dr[:, 2 * hp_:2 * hp_ + 2, bass.ts(ti_, ST)],
                )

            def _first_prefetches():
                # one pair ahead only: two-ahead would overwrite the slot the
                # current pair is still reading (same ring parity)
                prefetch_q(0)

            evac3_rest = None
            for st in range(nst):
                q_ps, k_ps, v_ps, sq = pass_b(
                    st, interleave0 if st == 0 else None)
                evac3_rest = evac(st, q_ps, k_ps, v_ps, sq,
                                  defer=(st == nst - 1))
                if st == nst - 2:
                    # q for the first two attention pairs is ready (tile 0);
                    # land the loads in tile 3's DMA slack
                    prefetch_q(0)
                    prefetch_q(1)

            # ---- attention + o_proj ----

            # wo per-head loads, slot-chained behind the wq quarters; split
            # in halves so they don't starve the x stream in tile 3
            wom = []
            for m in range(qh):
                t = persist.tile([128, h], F32R, tag=f"bigw{m}", name=f"wom{m}")
                nc.sync.dma_start(out=t[:, 0:h // 2], in_=wo_t[:, m, 0:h // 2])
                nc.sync.dma_start(out=t[:, h // 2:h], in_=wo_t[:, m, h // 2:h])
                wom.append(t)

            attnT_h = [
                persist.tile([128, 2, s], F32R, tag="wk", name="attnT01"),
                persist.tile([128, 2, s], F32R, tag="wv", name="attnT23"),
            ]
            # o_proj staging ring reuses the sin-table slot (sin's last read
            # is at the end of phase 1)
            out_sb = persist.tile([128, 2, h // 2], BF16, tag="sin",
                                  name="out_sb")

            def attn_slice(m, sl):
                return attnT_h[m // 2][:, m % 2, sl]

            def o_proj_chunk(sc):
                scs = bass.ts(sc, 128)
                for half in range(2):
                    ot = out_sb[:, (sc * 2 + half) % 2]
                    for hq in range(4):
                        ht = half * 4 + hq
                        o_ps = ps.tile([128, 512], F32, tag="pD", bufs=2,
                                       name="o_ps")
                        for m in range(qh):
                            nc.tensor.matmul(
                                o_ps, attn_slice(m, scs),
                                wom[m][:, bass.ts(ht, 512)],
                                start=(m == 0), stop=(m == qh - 1),
                            )
                        dst = ot[:, bass.ts(hq, 512)]
                        if (sc + ht) % 2 == 0:
                            nc.scalar.copy(dst, o_ps)
                        else:
                            nc.vector.tensor_copy(dst, o_ps)
                    nc.sync.dma_start(
                        out=out[scs, bass.ts(half, h // 2)], in_=ot
                    )

            # o_proj chunks are deferred one i-tile and dribbled out at the
            # attention seams (one per head-loop), where the ZR-evac chains
            # would otherwise leave PE idle
            pending_oproj = []

            def pop_filler(n=1):
                for _ in range(n):
                    if pending_oproj:
                        o_proj_chunk(pending_oproj.pop(0))

            def attn_tile(hp, ti):
                pi = ti * 2 + hp
                slot = pi % 2
                iss = bass.ts(ti, ST)
                if pi + 1 >= 2:
                    prefetch_q(pi + 1)
                njc = (ti + 1) * (ST // 128)
                # heads run sequentially: each gets the full z bank at
                # partition 0 (matmul dst must start at partition 0) and a
                # 2-deep scores pipeline in the three pA slots
                for i in range(2):
                    hh = 2 * hp + i
                    av = ps.tile([128, ST], F32, tag="pB", bufs=2,
                                 name=f"av{i}")
                    zz = ps.tile([1, ST], F32, tag="pC", bufs=1, name="zz")
                    sdict = {}

                    def emit_scores(jc):
                        i0 = min(max(jc * 128 - ti * ST, 0), ST - 256)
                        sp_ = ps.tile([128, ST], F32, tag="pA", bufs=3,
                                      name="s_ps")
                        nc.tensor.matmul(
                            sp_[:, i0:], kT_sb[:, bass.ts(jc, 128)],
                            qin_sb[:, slot, i, i0:], start=True, stop=True,
                        )
                        sdict[jc] = (sp_, i0)

                    for w in range(min(3, njc)):
                        emit_scores(w)
                    for jc in range(njc):
                        if jc + 3 < njc:
                            emit_scores(jc + 3)
                        diag = (jc + 1) * 128 > ti * ST
                        sp_, i0 = sdict.pop(jc)
                        p = probs.tile([128, ST], F32R, tag="p", name="p",
                                       bufs=4)
                        nc.scalar.activation(p[:, i0:], sp_[:, i0:], Exp,
                                             scale=scale)
                        if diag:
                            nc.gpsimd.affine_select(
                                out=p[:, i0:], in_=p[:, i0:],
                                pattern=[[1, ST - i0]],
                                compare_op=mybir.AluOpType.is_ge,
                                fill=0.0,
                                base=ti * ST + i0 - jc * 128,
                                channel_multiplier=-1,
                            )
                        st_, sp_f = (jc == 0), (jc == njc - 1)
                        nc.tensor.matmul(av[:, i0:], vnat_sb[:, jc, :],
                                         p[:, i0:], start=st_, stop=sp_f)
                        nc.tensor.matmul(zz[0:1, i0:], ones_r,
                                         p[:, i0:], start=st_, stop=sp_f)
                    zr = statp.tile([1, ST], F32, tag="zrs", name="zr")
                    nc.vector.reciprocal(zr, zz[0:1, :])
                    ZR = zrp.tile([128, ST], F32, tag="zr", name="ZR")
                    nc.gpsimd.partition_broadcast(ZR, zr)
                    nc.vector.tensor_mul(attn_slice(hh, iss), av, ZR)

            for ti in range(nst):
                for hp in range(2):
                    attn_tile(hp, ti)
                    if evac3_rest is not None:
                        # the last tile's rope/v work only feeds the
                        # ti=3 pairs; emit it behind the first pair
                        evac3_rest()
                        evac3_rest = None
                    pop_filler(2)
                pending_oproj.extend(
                    range(ti * (ST // 128), (ti + 1) * (ST // 128)))
            while pending_oproj:
                o_proj_chunk(pending_oproj.pop(0))

    nc.compile()
    return nc


def make_core_inputs(hidden_states, cos, sin, norm_w, wq, wk, wv, wo,
                     s=S, h=H, qh=QH, n_cores=N_CORES):
    """Host-side sharding + layout preparation. Returns list of in_maps."""
    dq = qh * HD
    dkv = DKV
    x = np.asarray(hidden_states, dtype=np.float32).reshape(s, h)
    nw = np.asarray(norm_w, dtype=np.float32)
    xT = np.ascontiguousarray(x.T)                      # [h, s]
    cosT = np.ascontiguousarray(np.asarray(cos, np.float32).reshape(s, HD).T)
    sinT = np.ascontiguousarray(np.asarray(sin, np.float32).reshape(s, HD).T)
    # swapped/sign-flipped sin table: rows 0:64 = +sin_half, 64:128 = -sin_half
    sin_half = sinT[0:64]
    sinTs = np.ascontiguousarray(np.concatenate([sinT[64:128], -sin_half], axis=0))
    # fold norm_w into the projection weights
    wq_f = np.asarray(wq, np.float32) * nw[:, None]
    wk_f = np.asarray(wk, np.float32) * nw[:, None]
    wv_f = np.asarray(wv, np.float32) * nw[:, None]
    wo_f = np.asarray(wo, np.float32)

    in_maps = []
    for c in range(n_cores):
        in_maps.append({
            "xT": xT,
            "wq": np.ascontiguousarray(wq_f[:, c * dq:(c + 1) * dq]),
            "wk": np.ascontiguousarray(wk_f[:, c * dkv:(c + 1) * dkv]),
            "wv": np.ascontiguousarray(wv_f[:, c * dkv:(c + 1) * dkv]),
            "wo": np.ascontiguousarray(wo_f[c * dq:(c + 1) * dq, :]),
            "cosT": cosT,
            "sinTs": sinTs,
        })
    return in_maps


_NC_CACHE = {}


def kernel(hidden_states, cos, sin, norm_w, wq, wk, wv, wo):
    from concourse.bass_utils import run_bass_kernel_spmd

    if "nc" not in _NC_CACHE:
        _NC_CACHE["nc"] = build_bass()
    nc = _NC_CACHE["nc"]
    in_maps = make_core_inputs(hidden_states, cos, sin, norm_w, wq, wk, wv, wo)
    res = run_bass_kernel_spmd(nc, in_maps, core_ids=list(range(N_CORES)))
    out = np.asarray(hidden_states, np.float32).reshape(S, H).copy()
    for m in res.results:
        out += np.asarray(m["out"], dtype=np.float32)
    return out.reshape(B, S, H)


# revision 54
# speedup vs baseline: 1.3449x; 1.0044x over previous
"""Mixtral attention layer (B=1, S=2048, H=4096, NH=32, NKV=8, HD=128) on 8
Trainium2 NeuronCores, tensor-parallel over heads.

Sharding: core c owns 4 query heads + 1 KV head (column-shard of wq/wk/wv,
row-shard of wo).  Each core computes a full [S, H] partial of the o_proj
output (bf16); the host sums the 8 partials and adds the residual.

Per-core pipeline (projection/attention matmuls in float32r = fp22-truncated
fp32, full PE rate at N>=256):
  Phase 1 (per 512-wide s-tile): x^T streamed fp32r once; per H-chunk, 6
    accumulating matmuls produce q^T (4 heads) / k^T / v^T in PSUM, and the
    norm stats ride the same stream (ACT squares into fp8e4; a dual-fp8
    DoubleRow ones-matmul reduces sum(x^2) at 2x PE rate).  PSUM evacuation
    is split: plain copies first on DVE (k,v) + ACT (q0-3) free the banks
    for the next tile with minimal latency; then r = 1/sqrt(mean+eps) is
    folded into the RoPE tables and applied SBUF-side on DVE while the next
    tile's matmuls run; v is transposed to natural [j, d] layout (PE
    transpose).  The last tile's table/rope/v work is deferred behind the
    first attention pair (only the ti=3 pairs consume it).  Weight DMAs are
    split fine (wk/wv quarters, wq 2-3 chunk pieces) and priority-ordered in
    need order so PE starts ~5us in; wo per-head loads chain into the wq
    slots and split in halves to avoid starving the tile-3 x stream.
  Attention: per (head-pair, i-tile), heads sequential, causal flash-style
    with a software-pipelined j-loop: scores run 2-3 chunks ahead of AV/Z
    (PSUM tags: scores x3, AV x2, Z x1, o_proj x2 = 8 banks; matmul dst
    must start at partition 0, hence one z bank per head in turn).  exp on
    ACT, causal mask via GPSIMD affine_select on diagonal chunks only, with
    free-dim subranges (>=256 wide, keeping fp32r full rate) on diagonal
    chunks to skip dead work.  Row-sum Z via ones-matmul accumulates
    alongside AV; 1/Z applied at AV evacuation into SBUF attn^T.
  o_proj: attn^T @ wo accumulated over the 4 heads, staged to bf16 rows and
    DMA'd out per 128-row half-chunk; chunks are deferred one i-tile and
    dribbled out at the attention seams so the matmuls fill the ZR-chain
    and head-switch stalls.

q^T is spilled to internal DRAM between phases (SBUF pressure) and
prefetched one head-pair ahead during attention (the 2-slot ring parity
forbids deeper prefetch).  Slot reuse chains: wq quarter -> wo per-head
(2MB x4), wk -> attnT heads 0-1, wv -> attnT 2-3, sin -> o_proj staging.
"""

import math

import numpy as np

import concourse.bass as bass
import concourse.tile as tile
from concourse import bacc, mybir
from concourse.masks import make_identity

F32 = mybir.dt.float32
F32R = mybir.dt.float32r
BF16 = mybir.dt.bfloat16
FP8 = mybir.dt.float8e4

# Full problem dims
B, S, H, NH, NKV, HD = 1, 2048, 4096, 32, 8, 128
EPS = 1e-5
N_CORES = 8
QH = NH // N_CORES          # query heads per core = 4
DQ = QH * HD                # q columns per core = 512
DKV = (NKV // N_CORES) * HD  # kv columns per core = 128

ST = 512                    # s-tile width
NST = S // ST               # 4
HC = H // 128               # 32 H-contraction chunks
G = 2                       # x chunks per DMA / squares pair


def build_bass(s=S, h=H, qh=QH):
    dq = qh * HD
    nst = s // ST
    scale = 1.0 / math.sqrt(HD)
    Exp = mybir.ActivationFunctionType.Exp

    nc = bacc.Bacc(None, target_bir_lowering=False)

    xT = nc.dram_tensor("xT", [h, s], F32R, kind="ExternalInput")
    wq = nc.dram_tensor("wq", [h, dq], F32R, kind="ExternalInput")
    wk = nc.dram_tensor("wk", [h, DKV], F32R, kind="ExternalInput")
    wv = nc.dram_tensor("wv", [h, DKV], F32R, kind="ExternalInput")
    wo = nc.dram_tensor("wo", [dq, h], F32R, kind="ExternalInput")
    cosT = nc.dram_tensor("cosT", [HD, s], F32, kind="ExternalInput")
    sinTs = nc.dram_tensor("sinTs", [HD, s], F32, kind="ExternalInput")
    out = nc.dram_tensor("out", [s, h], BF16, kind="ExternalOutput")

    xT_t = xT.rearrange("(ho hi) s -> hi ho s", hi=128)
    wq_t = wq.rearrange("(ho hi) d -> hi ho d", hi=128)
    wk_t = wk.rearrange("(ho hi) d -> hi ho d", hi=128)
    wv_t = wv.rearrange("(ho hi) d -> hi ho d", hi=128)
    wo_t = wo.rearrange("(do di) h -> di do h", di=128)

    with tile.TileContext(nc) as tc:
        with (
            tc.tile_pool(name="persist", bufs=1) as persist,
            tc.tile_pool(name="xin", bufs=4) as xin,
            tc.tile_pool(name="x2p", bufs=2) as x2p,
            tc.tile_pool(name="ropep", bufs=2) as ropep,
            tc.tile_pool(name="tabp", bufs=2) as tabp,
            tc.tile_pool(name="statp", bufs=2) as statp,
            tc.tile_pool(name="zrp", bufs=2) as zrp,
            tc.tile_pool(name="probs", bufs=4) as probs,
            tc.tile_pool(name="dramp", bufs=1, space="DRAM") as dramp,
            tc.tile_pool(name="acc_ps", bufs=8, space="PSUM") as ps,
        ):
            # ---- persistent SBUF ----
            # PSUM tags (8 banks): pA(3) q0-2 | scores; pB(2) q3,k | av;
            # pC(1) sum-sq | z-pair; pD(2) v,v-transpose | o_proj.
            wqq = [
                persist.tile([128, 8, dq], F32R, tag=f"bigw{i}", name=f"wqq{i}")
                for i in range(4)
            ]
            wk_sb = persist.tile([128, HC, DKV], F32R, tag="wk")
            wv_sb = persist.tile([128, HC, DKV], F32R, tag="wv")
            cos_sb = persist.tile([128, s], F32, tag="cos")
            sin_sb = persist.tile([128, s], F32, tag="sin")
            kT_sb = persist.tile([128, s], F32R, tag="kT")
            vT_sb = persist.tile([128, s], F32, tag="vT")
            vnat_sb = persist.tile([128, s // 128, 128], F32R, tag="vnat")
            qin_sb = persist.tile([128, 2, 2, ST], F32R, tag="qin")
            ones_f = persist.tile([128, 1], F32, tag="ones_f")
            ones_r = persist.tile([128, 1], F32R, tag="ones_r")
            ones_b = persist.tile([128, 1], BF16, tag="ones_b")
            ones8 = persist.tile([128, 2, 128], FP8, tag="ones8")
            ones128_f = persist.tile([128, 128], F32, tag="ones128")
            eps_sb = persist.tile([1, 1], F32, tag="eps")
            ident_sb = persist.tile([128, 128], F32, tag="ident")
            qT_dr = dramp.tile([128, qh, s], F32R, tag="qT_dr")

            nc.vector.memset(ones_f, 1.0)

            nc.scalar.copy(ones_b, ones_f)
            nc.vector.memset(ones128_f, 1.0)
            nc.scalar.copy(ones_r, ones128_f[:, 0:1])
            nc.scalar.copy(ones8[:, 0, :], ones128_f)
            nc.scalar.copy(ones8[:, 1, :], ones128_f)
            nc.vector.memset(eps_sb, EPS)
            make_identity(nc, ident_sb)

            # first-needed weight pieces lead the DMA queue; the x stream and
            # the weight tails interleave behind them (priority = emission)
            nc.sync.dma_start(out=wk_sb[:, 0:4, :], in_=wk_t[:, 0:4, :])
            nc.sync.dma_start(out=wv_sb[:, 0:4, :], in_=wv_t[:, 0:4, :])
            nc.sync.dma_start(out=wqq[0][:, 0:2, :], in_=wq_t[:, 0:2, :])

            # ---- phase 1 ----
            def pass_b(st, interleave=None):
                ss = bass.ts(st, ST)
                q_ps = [
                    ps.tile([128, ST], F32, tag=("pA" if m < 3 else "pB"),
                            bufs=(3 if m < 3 else 2), name=f"q_ps{m}")
                    for m in range(qh)
                ]
                k_ps = ps.tile([128, ST], F32, tag="pB", bufs=2, name="k_ps")
                v_ps = ps.tile([128, ST], F32, tag="pD", bufs=2, name="v_ps")
                sq = ps.tile([128, ST], F32, tag="pC", bufs=1, name="sq")
                for g in range(HC // G):
                    xg = xin.tile([128, G, ST], F32R, name="xg")
                    nc.sync.dma_start(out=xg, in_=xT_t[:, g * G:(g + 1) * G, ss])
                    x2 = x2p.tile([128, 2, ST], FP8, name="x2")
                    for cc in range(G):
                        hc = g * G + cc
                        x_sb = xg[:, cc, :]
                        st_, sp_ = (hc == 0), (hc == HC - 1)
                        qt, lo = hc // 8, hc % 8
                        nc.tensor.matmul(k_ps, wk_sb[:, hc, :], x_sb,
                                         start=st_, stop=sp_)
                        nc.tensor.matmul(v_ps, wv_sb[:, hc, :], x_sb,
                                         start=st_, stop=sp_)
                        for m in range(qh):
                            nc.tensor.matmul(
                                q_ps[m], wqq[qt][:, lo, bass.ts(m, 128)], x_sb,
                                start=st_, stop=sp_,
                            )
                        nc.scalar.square(x2[:, cc, :], x_sb)
                    # all-ones weight is invariant under the SwInterleave
                    # layout, so the 2x-rate dual-fp8 mode needs no repacking
                    nc.tensor.matmul(
                        sq, ones8, x2,
                        start=(g == 0), stop=(g == HC // G - 1),
                        perf_mode=mybir.MatmulPerfMode.DoubleRow,
                    )
                    if interleave is not None:
                        interleave(g)
                return q_ps, k_ps, v_ps, sq

            def evac(st, q_ps, k_ps, v_ps, sq, defer=False):
                ss = bass.ts(st, ST)
                # plain copies first, split ACT/DVE in the next tile's
                # chunk-0 matmul order (k,v,q0..q3): frees the PSUM banks
                # with minimal serial latency
                nc.vector.tensor_copy(kT_sb[:, ss], k_ps)
                nc.vector.tensor_copy(vT_sb[:, ss], v_ps)
                qds = []
                for m in range(qh):
                    qd = ropep.tile([128, ST], F32R, tag="qd", name="qd",
                                    bufs=4)
                    nc.scalar.copy(qd, q_ps[m])
                    qds.append(qd)
                # stats scalars immediately (frees the pC bank + stat slots)
                sd = statp.tile([1, ST], F32, tag="stat", name="sd")
                nc.scalar.activation(
                    sd, sq[0:1, :], mybir.ActivationFunctionType.Sqrt,
                    bias=eps_sb, scale=1.0 / h,
                )
                rr = statp.tile([1, ST], F32, tag="stat", name="rr")
                nc.vector.reciprocal(rr, sd)

                def rest():
                    # rope tables, then SBUF-side rotation on DVE
                    R = tabp.tile([128, ST], F32, tag="R", name="R")
                    nc.gpsimd.partition_broadcast(R, rr)
                    cp = tabp.tile([128, ST], F32, tag="cp", name="cp")
                    nc.vector.tensor_mul(cp, cos_sb[:, ss], R)
                    sp = tabp.tile([128, ST], F32, tag="sp", name="sp")
                    nc.vector.tensor_mul(sp, sin_sb[:, ss], R)
                    # v path first: the transposes recycle the pC bank for
                    # the next tile's stats accumulator (and the z-pair)
                    nc.vector.tensor_mul(vT_sb[:, ss], vT_sb[:, ss], R)
                    for j4 in range(ST // 128):
                        jc = st * (ST // 128) + j4
                        vt = ps.tile([128, 128], F32, tag="pC", bufs=1,
                                     name="vt")
                        nc.tensor.transpose(vt, vT_sb[:, bass.ts(jc, 128)],
                                            ident_sb)
                        nc.scalar.copy(vnat_sb[:, jc, :], vt)

                    def rope(dst):
                        u = ropep.tile([128, ST], F32, tag="u", name="u",
                                       bufs=1)
                        nc.vector.tensor_mul(u[0:64, :], dst[64:128, :],
                                             sp[64:128, :])
                        nc.vector.tensor_mul(u[64:128, :], dst[0:64, :],
                                             sp[0:64, :])
                        nc.vector.tensor_mul(dst, dst, cp)
                        nc.vector.tensor_add(dst, dst, u)

                    for m in range(qh):
                        rope(qds[m])
                        nc.sync.dma_start(out=qT_dr[:, m, ss], in_=qds[m])
                    rope(kT_sb[:, ss])

                if defer:
                    return rest
                rest()
                return None

            def _wq_piece(qt, a, b):
                nc.sync.dma_start(out=wqq[qt][:, a:b, :],
                                  in_=wq_t[:, 8 * qt + a:8 * qt + b, :])

            def _kv_piece(a, b):
                nc.sync.dma_start(out=wk_sb[:, a:b, :], in_=wk_t[:, a:b, :])
                nc.sync.dma_start(out=wv_sb[:, a:b, :], in_=wv_t[:, a:b, :])

            # need-ordered weight stream: one piece after each x group of
            # tile 0 (tile 0 is DMA-paced; later tiles have DMA slack)
            _pieces = [
                lambda: (_kv_piece(4, 8), _wq_piece(0, 2, 4)),
                lambda: _wq_piece(0, 4, 8),
                lambda: _kv_piece(8, 16),
                lambda: _wq_piece(1, 0, 3),
                lambda: _wq_piece(1, 3, 6),
                lambda: _wq_piece(1, 6, 8),
                lambda: _kv_piece(16, 24),
                lambda: _wq_piece(2, 0, 3),
                lambda: _wq_piece(2, 3, 6),
                lambda: _wq_piece(2, 6, 8),
                lambda: _kv_piece(24, 32),
                lambda: _wq_piece(3, 0, 3),
                lambda: _wq_piece(3, 3, 6),
                lambda: _wq_piece(3, 6, 8),
                lambda: nc.sync.dma_start(out=cos_sb, in_=cosT[:, :]),
                lambda: nc.sync.dma_start(out=sin_sb, in_=sinTs[:, :]),
            ]

            def interleave0(g):
                if g < len(_pieces):
                    _pieces[g]()

            def prefetch_q(pi):
                if pi >= 2 * nst:
                    return
                ti_, hp_ = pi // 2, pi % 2
                nc.sync.dma_start(
                    out=qin_sb[:, pi % 2],
                    in_=qT_dr[:, 2 * hp_:2 * hp_ + 2, bass.ts(ti_, ST)],
                )

            def _first_prefetches():
                # one pair ahead only: two-ahead would overwrite the slot the
                # current pair is still reading (same ring parity)
                prefetch_q(0)

            evac3_rest = None
            for st in range(nst):
                q_ps, k_ps, v_ps, sq = pass_b(
                    st, interleave0 if st == 0 else None)
                evac3_rest = evac(st, q_ps, k_ps, v_ps, sq,
                                  defer=(st == nst - 1))
                if st == nst - 2:
                    # q for the first two attention pairs is ready (tile 0);
                    # land the loads in tile 3's DMA slack
                    prefetch_q(0)
                    prefetch_q(1)

            # ---- attention + o_proj ----

            # wo per-head loads, slot-chained behind the wq quarters; split
            # in halves so they don't starve the x stream in tile 3
            wom = []
            for m in range(qh):
                t = persist.tile([128, h], F32R, tag=f"bigw{m}", name=f"wom{m}")
                nc.sync.dma_start(out=t[:, 0:h // 2], in_=wo_t[:, m, 0:h // 2])
                nc.sync.dma_start(out=t[:, h // 2:h], in_=wo_t[:, m, h // 2:h])
                wom.append(t)

            attnT_h = [
                persist.tile([128, 2, s], F32R, tag="wk", name="attnT01"),
                persist.tile([128, 2, s], F32R, tag="wv", name="attnT23"),
            ]
            # o_proj staging ring reuses the sin-table slot (sin's last read
            # is at the end of phase 1)
            out_sb = persist.tile([128, 2, h // 2], BF16, tag="sin",
                                  name="out_sb")

            def attn_slice(m, sl):
                return attnT_h[m // 2][:, m % 2, sl]

            def o_proj_chunk(sc):
                scs = bass.ts(sc, 128)
                for half in range(2):
                    ot = out_sb[:, (sc * 2 + half) % 2]
                    for hq in range(4):
                        ht = half * 4 + hq
                        o_ps = ps.tile([128, 512], F32, tag="pD", bufs=2,
                                       name="o_ps")
                        for m in range(qh):
                            nc.tensor.matmul(
                                o_ps, attn_slice(m, scs),
                                wom[m][:, bass.ts(ht, 512)],
                                start=(m == 0), stop=(m == qh - 1),
                            )
                        # DVE only: ACT must stay clear for the exp chain
                        # that paces the attention j-loops
                        dst = ot[:, bass.ts(hq, 512)]
                        nc.vector.tensor_copy(dst, o_ps)
                    nc.sync.dma_start(
                        out=out[scs, bass.ts(half, h // 2)], in_=ot
                    )

            # o_proj chunks are deferred one i-tile and dribbled out at the
            # attention seams (one per head-loop), where the ZR-evac chains
            # would otherwise leave PE idle
            pending_oproj = []

            def pop_filler(n=1):
                for _ in range(n):
                    if pending_oproj:
                        o_proj_chunk(pending_oproj.pop(0))

            def attn_tile(hp, ti):
                pi = ti * 2 + hp
                slot = pi % 2
                iss = bass.ts(ti, ST)
                if pi + 1 >= 2:
                    prefetch_q(pi + 1)
                njc = (ti + 1) * (ST // 128)
                # heads run sequentially: each gets the full z bank at
                # partition 0 (matmul dst must start at partition 0) and a
                # 2-deep scores pipeline in the three pA slots
                for i in range(2):
                    hh = 2 * hp + i
                    av = ps.tile([128, ST], F32, tag="pB", bufs=2,
                                 name=f"av{i}")
                    zz = ps.tile([1, ST], F32, tag="pC", bufs=1, name="zz")
                    sdict = {}

                    def emit_scores(jc):
                        i0 = min(max(jc * 128 - ti * ST, 0), ST - 256)
                        sp_ = ps.tile([128, ST], F32, tag="pA", bufs=3,
                                      name="s_ps")
                        nc.tensor.matmul(
                            sp_[:, i0:], kT_sb[:, bass.ts(jc, 128)],
                            qin_sb[:, slot, i, i0:], start=True, stop=True,
                        )
                        sdict[jc] = (sp_, i0)

                    for w in range(min(3, njc)):
                        emit_scores(w)
                    for jc in range(njc):
                        if jc + 3 < njc:
                            emit_scores(jc + 3)
                        diag = (jc + 1) * 128 > ti * ST
                        sp_, i0 = sdict.pop(jc)
                        p = probs.tile([128, ST], F32R, tag="p", name="p",
                                       bufs=4)
                        nc.scalar.activation(p[:, i0:], sp_[:, i0:], Exp,
                                             scale=scale)
                        if diag:
                            nc.gpsimd.affine_select(
                                out=p[:, i0:], in_=p[:, i0:],
                                pattern=[[1, ST - i0]],
                                compare_op=mybir.AluOpType.is_ge,
                                fill=0.0,
                                base=ti * ST + i0 - jc * 128,
                                channel_multiplier=-1,
                            )
                        st_, sp_f = (jc == 0), (jc == njc - 1)
                        nc.tensor.matmul(av[:, i0:], vnat_sb[:, jc, :],
                                         p[:, i0:], start=st_, stop=sp_f)
                        nc.tensor.matmul(zz[0:1, i0:], ones_r,
                                         p[:, i0:], start=st_, stop=sp_f)
                    zr = statp.tile([1, ST], F32, tag="zrs", name="zr")
                    nc.vector.reciprocal(zr, zz[0:1, :])
                    ZR = zrp.tile([128, ST], F32, tag="zr", name="ZR")
                    nc.gpsimd.partition_broadcast(ZR, zr)
                    nc.vector.tensor_mul(attn_slice(hh, iss), av, ZR)

            for ti in range(nst):
                for hp in range(2):
                    attn_tile(hp, ti)
                    if evac3_rest is not None and (ti, hp) == (0, 1):
                        # the last tile's rope/v work only feeds the
                        # ti=3 pairs; emit it behind the second pair
                        evac3_rest()
                        evac3_rest = None
                    pop_filler(2)
                pending_oproj.extend(
                    range(ti * (ST // 128), (ti + 1) * (ST // 128)))
            while pending_oproj:
                o_proj_chunk(pending_oproj.pop(0))

    nc.compile()
    return nc


def make_core_inputs(hidden_states, cos, sin, norm_w, wq, wk, wv, wo,
                     s=S, h=H, qh=QH, n_cores=N_CORES):
    """Host-side sharding + layout preparation. Returns list of in_maps."""
    dq = qh * HD
    dkv = DKV
    x = np.asarray(hidden_states, dtype=np.float32).reshape(s, h)
    nw = np.asarray(norm_w, dtype=np.float32)
    xT = np.ascontiguousarray(x.T)                      # [h, s]
    cosT = np.ascontiguousarray(np.asarray(cos, np.float32).reshape(s, HD).T)
    sinT = np.ascontiguousarray(np.asarray(sin, np.float32).reshape(s, HD).T)
    # swapped/sign-flipped sin table: rows 0:64 = +sin_half, 64:128 = -sin_half
    sin_half = sinT[0:64]
    sinTs = np.ascontiguousarray(np.concatenate([sinT[64:128], -sin_half], axis=0))
    # fold norm_w into the projection weights
    wq_f = np.asarray(wq, np.float32) * nw[:, None]
    wk_f = np.asarray(wk, np.float32) * nw[:, None]
    wv_f = np.asarray(wv, np.float32) * nw[:, None]
    wo_f = np.asarray(wo, np.float32)

    in_maps = []
    for c in range(n_cores):
        in_maps.append({
            "xT": xT,
            "wq": np.ascontiguousarray(wq_f[:, c * dq:(c + 1) * dq]),
            "wk": np.ascontiguousarray(wk_f[:, c * dkv:(c + 1) * dkv]),
            "wv": np.ascontiguousarray(wv_f[:, c * dkv:(c + 1) * dkv]),
            "wo": np.ascontiguousarray(wo_f[c * dq:(c + 1) * dq, :]),
            "cosT": cosT,
            "sinTs": sinTs,
        })
    return in_maps


_NC_CACHE = {}


def kernel(hidden_states, cos, sin, norm_w, wq, wk, wv, wo):
    from concourse.bass_utils import run_bass_kernel_spmd

    if "nc" not in _NC_CACHE:
        _NC_CACHE["nc"] = build_bass()
    nc = _NC_CACHE["nc"]
    in_maps = make_core_inputs(hidden_states, cos, sin, norm_w, wq, wk, wv, wo)
    res = run_bass_kernel_spmd(nc, in_maps, core_ids=list(range(N_CORES)))
    out = np.asarray(hidden_states, np.float32).reshape(S, H).copy()
    for m in res.results:
        out += np.asarray(m["out"], dtype=np.float32)
    return out.reshape(B, S, H)
